# revision 26
# baseline (speedup 1.0000x reference)
"""Trainium2 Bass kernel for nn_BiLSTM_CRF (BiGRU + degenerate-CRF viterbi).

Key insight: the reference viterbi broadcasts prev over axis 0 and reduces over
axis 0, so new[j] = logit[j] + prev[j] + max_i trans[i,j] -- the recurrence
decouples across tags. Backpointers are time-constant A[j] = argmax_i
trans[i,j], path[t] = A^(L-t)(start), path[0] = 0, start = argmax_j of
(sum_t probs[t,j] + (L-1) * max_i trans[i,j]).

So the device pipeline is: embedding gather -> BiGRU (sequential, bf16
matmuls) -> logits softmax -> per-tag prob sums -> tiny map-iteration tail.
"""

import os

import numpy as np
import ml_dtypes

import concourse.bass as bass
import concourse.bacc as bacc
import concourse.tile as tile
from concourse import mybir
from concourse import bass_utils

V, E, H, T = 200000, 100, 128, 6
L = int(os.environ.get("KERNEL_L", "4096"))
UNROLL = int(os.environ.get("KERNEL_UNROLL", "0"))  # 0 = full static unroll
N_CORES = int(os.environ.get("KERNEL_CORES", "8"))

f32 = mybir.dt.float32
bf16 = mybir.dt.bfloat16
i32 = mybir.dt.int32
AF = mybir.ActivationFunctionType
OP = mybir.AluOpType
AX = mybir.AxisListType

_last_results = None


def _rev_free(ap):
    """Reverse the innermost free dim of a 2-D [P, W] AP slice."""
    w = ap.ap[-1][1]
    assert ap.ap[-1][0] == 1
    return bass.AP(tensor=ap.tensor, offset=ap.offset + (w - 1),
                   ap=[ap.ap[0], [-1, w]])


def _build(nc):
    # ---------------- DRAM I/O ----------------
    sent = nc.dram_tensor("sentence", [L], i32, kind="ExternalInput")
    emb = nc.dram_tensor("emb", [V, E], f32, kind="ExternalInput")
    # all small constants consolidated into two tensors -> two DMAs -> two
    # DMA-lane semaphores that each engine absorbs once (1-wait limit).
    cf_d = nc.dram_tensor("constf32", [H, 33], f32, kind="ExternalInput")
    cb_d = nc.dram_tensor("constbf", [H, 1678], bf16, kind="ExternalInput")

    path_out = nc.dram_tensor("path", [1, L + 1], i32, kind="ExternalOutput")
    score_out = nc.dram_tensor("score", [1, 1], f32, kind="ExternalOutput")

    NCH = L // 128          # gather / logits chunks
    NGI = L // 512          # gi matmul chunks

    with tile.TileContext(nc) as tc:
        with (
            tc.tile_pool(name="const", bufs=1) as cpool,
            tc.tile_pool(name="big", bufs=1) as big,
        ):
            # ------------- constants into SBUF -------------
            cf = cpool.tile([H, 33], f32)
            nc.sync.dma_start(out=cf[:], in_=cf_d[:, :])
            cb = cpool.tile([H, 1678], bf16)
            nc.sync.dma_start(out=cb[:], in_=cb_d[:, :])
            whhT = cb[:, 0:768]
            wihT = cb[:, 768:1536]
            woutT = cb[:, 1536:1548]
            id128 = cb[:, 1548:1676]
            h0bf = cb[:, 1676:1678]
            gibias = cf[:, 0:6]
            bhhn = cf[:, 6:8]
            transT = cf[0:T, 14:20]
            iota6 = cf[0:1, 20:26]
            iota6r = cf[0:T, 20:26]
            id6 = cf[0:T, 26:32]
            ones128 = cf[:, 32:33]
            # DVE-local copies: TT-class DVE ops can encode only ONE sem wait,
            # so any tile feeding a tensor_tensor op must be DVE-produced.
            iota6v = cpool.tile([1, T], f32)
            nc.vector.tensor_copy(iota6v[:], iota6)
            iota6rv = cpool.tile([T, T], f32)
            nc.vector.tensor_copy(iota6rv[:], iota6r)
            boutv = cpool.tile([H, T], f32)
            nc.vector.tensor_copy(boutv[:], cf[:, 8:14])


            # ------------- big persistent buffers -------------
            xT = big.tile([128, L], bf16)           # x^T (E on partitions)
            gi_f = big.tile([H, 3 * L], bf16)       # input projections fwd
            gi_b = big.tile([H, 3 * L], bf16)       # input proj bwd, TIME-REVERSED
            hf = big.tile([H, L + 1], bf16)         # fwd hidden states
            hbrev = big.tile([H, L + 1], bf16)      # bwd hidden, reversed order
            hblin = big.tile([H, L], bf16)          # bwd hidden, original order

            nc.vector.tensor_copy(hf[:, 0:1], h0bf[:, 0:1])
            nc.vector.tensor_copy(hbrev[:, 0:1], h0bf[:, 1:2])

            # ------------- phase A: gather + transpose -------------
            with (
                tc.tile_pool(name="gather", bufs=NCH) as gpool,
                tc.tile_pool(name="trps", bufs=2, space="PSUM") as trps,
            ):
                for c in range(NCH):
                    idx = gpool.tile([128, 1], i32)
                    nc.sync.dma_start(out=idx, in_=sent[c * 128:(c + 1) * 128, None])
                    xg = gpool.tile([128, E], f32)
                    nc.gpsimd.indirect_dma_start(
                        out=xg[:],
                        out_offset=None,
                        in_=emb[:, :],
                        in_offset=bass.IndirectOffsetOnAxis(ap=idx[:, :1], axis=0),
                    )
                    xgb = gpool.tile([128, E], bf16)
                    nc.vector.tensor_copy(xgb[:], xg[:])
                    pt = trps.tile([E, 128], bf16)
                    nc.tensor.transpose(out=pt[:], in_=xgb[:], identity=id128)
                    nc.vector.tensor_copy(xT[:E, c * 128:(c + 1) * 128], pt[:, :])

            # ------------- phase B: gi = x @ WihT (+bias) -------------
            with tc.tile_pool(name="gips", bufs=2, space="PSUM") as gips:
                for dg in range(6):          # (dir, gate)
                    d, g = divmod(dg, 3)
                    for n in range(NGI):
                        pg = gips.tile([H, 512], f32)
                        nc.tensor.matmul(
                            out=pg[:],
                            lhsT=wihT[:E, dg * H:(dg + 1) * H],
                            rhs=xT[:E, n * 512:(n + 1) * 512],
                            start=True, stop=True,
                        )
                        if d == 0:
                            dst = gi_f[:, g * L + n * 512: g * L + (n + 1) * 512]
                        else:
                            # bwd step s handles original time L-1-s: write
                            # time chunk [512n, 512n+512) to reversed position
                            dst = _rev_free(
                                gi_b[:, g * L + L - 512 * (n + 1):
                                     g * L + L - 512 * n])
                        if g == 2:
                            # n-gate input projection is stored NEGATED so the
                            # recurrence can compute -n on ACT (see gru_step)
                            nc.vector.tensor_scalar(
                                dst, pg[:], gibias[:, dg:dg + 1], -1.0,
                                OP.add, OP.mult,
                            )
                        else:
                            nc.vector.tensor_scalar(
                                dst, pg[:], gibias[:, dg:dg + 1], None, OP.add,
                            )

            # ------------- phase C: the BiGRU recurrence -------------
            with (
                tc.tile_pool(name="psF", bufs=2, space="PSUM") as psF,
                tc.tile_pool(name="psB", bufs=2, space="PSUM") as psB,
                tc.tile_pool(name="step", bufs=4) as sp,
            ):
                def gru_step(t_f, whh_off, gi, hbuf, ps_pool, bhh_col, rd, wr):
                    """One GRU step; rd/wr are h-buffer column indices."""
                    ps = ps_pool.tile([H, 3], f32, tag="ps")
                    for g in range(3):
                        nc.tensor.matmul(
                            out=ps[:, g:g + 1],
                            lhsT=whhT[:, whh_off + g * H: whh_off + (g + 1) * H],
                            rhs=hbuf[:, rd],
                            start=True, stop=True,
                        )
                    rz = sp.tile([H, 2], f32, tag="rz")
                    nc.scalar.activation(rz[:, 0:1], ps[:, 0:1], AF.Sigmoid,
                                         bias=gi[:, bass.ds(t_f, 1)])
                    nc.scalar.activation(rz[:, 1:2], ps[:, 1:2], AF.Sigmoid,
                                         bias=gi[:, bass.ds(L + t_f, 1)])
                    tmp = sp.tile([H, 1], f32, tag="tmp")
                    nc.vector.tensor_scalar(tmp[:], ps[:, 2:3],
                                            bhhn[:, bhh_col:bhh_col + 1],
                                            rz[:, 0:1], OP.add, OP.mult)
                    # gi_n is stored negated, so this yields -n = tanh(-tmp-i_n)
                    nneg = sp.tile([H, 1], f32, tag="nneg")
                    nc.scalar.activation(nneg[:], tmp[:], AF.Tanh,
                                         bias=gi[:, bass.ds(2 * L + t_f, 1)],
                                         scale=-1.0)
                    # dd = h - n on ACT (keeps DVE at 2 ops/step/dir)
                    dd = sp.tile([H, 1], f32, tag="dd")
                    nc.scalar.activation(dd[:], hbuf[:, rd], AF.Identity,
                                         bias=nneg[:, 0:1])
                    # h' = z*(h-n) + n = z*dd - nneg
                    nc.vector.tensor_scalar(hbuf[:, wr], dd[:],
                                            rz[:, 1:2], nneg[:, 0:1],
                                            OP.mult, OP.subtract)

                def both_steps(t):
                    # forward step t: h[t] -> h[t+1]
                    gru_step(t, 0, gi_f, hf, psF, 0,
                             bass.ds(t, 1), bass.ds(t + 1, 1))
                    # backward step t processes original time L-1-t, but
                    # all its buffers are stored time-reversed, so it
                    # indexes exactly like the forward direction.
                    gru_step(t, 3 * H, gi_b, hbrev, psB, 1,
                             bass.ds(t, 1), bass.ds(t + 1, 1))

                if UNROLL == 0:
                    for t in range(L):      # full static unroll
                        both_steps(t)
                else:
                    with tc.For_i(0, L, UNROLL,
                                  hint_engines=tuple(mybir.ALL_ENGINES)) as iv:
                        for k in range(UNROLL):
                            both_steps(iv + k)
            # restore original time order: h_b[t] = hbrev[:, L - t]
            nc.vector.tensor_copy(hblin[:], _rev_free(hbrev[:, 1:L + 1]))

            # ------------- phase D: logits, softmax, tag sums -------------
            Pacc = big.tile([H, T], f32)
            nc.vector.memset(Pacc[:], 0.0)
            with (
                tc.tile_pool(name="lps", bufs=2, space="PSUM") as lps,
                tc.tile_pool(name="lsb", bufs=3) as lsb,
            ):
                for c in range(NCH):
                    pl = lps.tile([H, T], f32)
                    nc.tensor.matmul(out=pl[:], lhsT=hf[:, 1 + c * 128: 1 + (c + 1) * 128],
                                     rhs=woutT[:, 0:T], start=True, stop=False)
                    nc.tensor.matmul(out=pl[:], lhsT=hblin[:, c * 128:(c + 1) * 128],
                                     rhs=woutT[:, T:2 * T], start=False, stop=True)
                    lg = lsb.tile([H, T], f32, tag="lg")
                    nc.vector.tensor_add(lg[:], pl[:], boutv[:])
                    ex = lsb.tile([H, T], f32, tag="ex")
                    se = lsb.tile([H, 1], f32, tag="se")
                    nc.scalar.activation(ex[:], lg[:], AF.Exp, accum_out=se[:])
                    rec = lsb.tile([H, 1], f32, tag="rec")
                    nc.vector.reciprocal(rec[:], se[:])
                    pb = lsb.tile([H, T], f32, tag="pb")
                    nc.vector.tensor_scalar(pb[:], ex[:], rec[:], None, OP.mult)
                    nc.vector.tensor_add(Pacc[:], Pacc[:], pb[:])

            # partition reduce S[j] = sum_t probs[t, j] via ones-matmul
            with tc.tile_pool(name="sps", bufs=1, space="PSUM") as sps:
                pS = sps.tile([1, T], f32)
                nc.tensor.matmul(out=pS[:], lhsT=ones128, rhs=Pacc[:],
                                 start=True, stop=True)
                S6 = big.tile([1, T], f32)
                nc.vector.tensor_copy(S6[:], pS[:])

            # ------------- phase E: M, A from transitions -------------
            Mcol = big.tile([T, 1], f32)
            nc.vector.tensor_reduce(Mcol[:], transT, AX.X, OP.max)
            eqT = big.tile([T, T], f32)
            nc.vector.tensor_scalar(eqT[:], transT, Mcol[:, 0:1], None, OP.is_equal)
            junk = big.tile([T, T], f32)
            nc.vector.tensor_mul(junk[:], eqT[:], iota6rv[:])
            Acol = big.tile([T, 1], f32)
            nc.vector.tensor_reduce(Acol[:], junk[:], AX.X, OP.max)
            with tc.tile_pool(name="maps", bufs=2, space="PSUM") as maps:
                pm = maps.tile([1, T], f32, tag="pm")
                nc.tensor.transpose(out=pm[:], in_=Mcol[:], identity=id6)
                Mf = big.tile([1, T], f32)
                nc.vector.tensor_copy(Mf[:], pm[:])
                pa = maps.tile([1, T], f32, tag="pa")
                nc.tensor.transpose(out=pa[:], in_=Acol[:], identity=id6)
                Af = big.tile([1, T], f32)
                nc.vector.tensor_copy(Af[:], pa[:])

            # ------------- phase F: score and start -------------
            fin = big.tile([1, T], f32)
            nc.vector.tensor_scalar(fin[:], Mf[:], float(L - 1), None, OP.mult)
            nc.vector.tensor_add(fin[:], fin[:], S6[:])
            sc = big.tile([1, 1], f32)
            nc.vector.tensor_reduce(sc[:], fin[:], AX.X, OP.max)
            eqf = big.tile([1, T], f32)
            nc.vector.tensor_scalar(eqf[:], fin[:], sc[:, 0:1], None, OP.is_equal)
            junk1 = big.tile([1, T], f32)
            nc.vector.tensor_mul(junk1[:], eqf[:], iota6v[:])
            st = big.tile([1, 1], f32)
            nc.vector.tensor_reduce(st[:], junk1[:], AX.X, OP.max)
            nc.sync.dma_start(out=score_out[:, :], in_=sc[:])

            # ------------- phase G: path via leftward doubling -------------
            # B[L-k] = A^k(start) for k=1..L-1; path[t] = B[t] for t=1..L-1.
            B = big.tile([1, L + 1], f32)
            C = big.tile([1, T], f32)       # C = A^m as a table
            nc.vector.tensor_copy(C[:], Af[:])
            # B[L-1] = A[start]
            selp = big.tile([1, T], f32)
            nc.vector.tensor_scalar(selp[:], iota6v[:], st[:, 0:1], None, OP.is_equal)
            junk2 = big.tile([1, T], f32)
            nc.vector.tensor_mul(junk2[:], selp[:], C[:])
            nc.vector.tensor_reduce(B[:, L - 1:L], junk2[:], AX.X, OP.max)
            with tc.tile_pool(name="dbl", bufs=2) as dbl:
                m = 1
                while m < L - 1:
                    ext = min(m, L - 1 - m)
                    # B[L-m-i] = C[B[L-i]] for i=1..ext
                    src = B[:, L - ext:L]
                    acc = dbl.tile([1, ext], f32, tag="acc")
                    nc.vector.memset(acc[:], 0.0)
                    for j in range(T):
                        mj = dbl.tile([1, ext], f32, tag="mj")
                        nc.vector.tensor_scalar(mj[:], src, float(j), None,
                                                OP.is_equal)
                        nc.vector.tensor_scalar(mj[:], mj[:], C[:, j:j + 1], None,
                                                OP.mult)
                        nc.vector.tensor_add(acc[:], acc[:], mj[:])
                    nc.vector.tensor_copy(B[:, L - m - ext:L - m], acc[:])
                    if 2 * m < L - 1:
                        # C = C o C
                        c2 = dbl.tile([1, T], f32, tag="c2")
                        nc.vector.memset(c2[:], 0.0)
                        for j in range(T):
                            cj = dbl.tile([1, T], f32, tag="cj")
                            nc.vector.tensor_scalar(cj[:], C[:], float(j), None,
                                                    OP.is_equal)
                            nc.vector.tensor_scalar(cj[:], cj[:], C[:, j:j + 1],
                                                    None, OP.mult)
                            nc.vector.tensor_add(c2[:], c2[:], cj[:])
                        nc.vector.tensor_copy(C[:], c2[:])
                    m += ext

            # ------------- phase H: assemble path -------------
            pf32 = big.tile([1, L + 1], f32)
            nc.vector.memset(pf32[:, 0:1], 0.0)
            nc.vector.tensor_copy(pf32[:, 1:L], B[:, 1:L])
            nc.vector.tensor_copy(pf32[:, L:L + 1], st[:])
            pi32 = big.tile([1, L + 1], i32)
            nc.vector.tensor_copy(pi32[:], pf32[:])
            nc.sync.dma_start(out=path_out[:, :], in_=pi32[:])

    return nc


def _prep_inputs(inputs):
    bf = ml_dtypes.bfloat16
    sent = np.ascontiguousarray(np.asarray(inputs["sentence"]).astype(np.int32))
    emb = np.ascontiguousarray(np.asarray(inputs["emb"], dtype=np.float32))
    h0 = np.asarray(inputs["h0"], dtype=np.float32)

    # constf32 [128, 33]: gibias(0:6) bhhn(6:8) boutrep(8:14) transT(14:20)
    # iota6rep(20:26) ident6(26:32) ones(32:33)
    cf = np.zeros((H, 33), np.float32)
    for d, (bih, bhh) in enumerate(
        [(inputs["b_ih_f"], inputs["b_hh_f"]), (inputs["b_ih_b"], inputs["b_hh_b"])]
    ):
        bih = np.asarray(bih, dtype=np.float32)
        bhh = np.asarray(bhh, dtype=np.float32)
        cf[:, d * 3 + 0] = bih[0:H] + bhh[0:H]
        cf[:, d * 3 + 1] = bih[H:2 * H] + bhh[H:2 * H]
        cf[:, d * 3 + 2] = bih[2 * H:3 * H]
        cf[:, 6 + d] = bhh[2 * H:3 * H]
    cf[:, 8:14] = np.asarray(inputs["b_out"], dtype=np.float32)[None, :]
    cf[0:T, 14:20] = np.asarray(inputs["transitions"], dtype=np.float32).T
    cf[0:T, 20:26] = np.arange(T, dtype=np.float32)[None, :]
    cf[0:T, 26:32] = np.eye(T, dtype=np.float32)
    cf[:, 32] = 1.0

    # constbf [128, 1678]: whhT(0:768) wihT(768:1536) woutT(1536:1548)
    # ident128(1548:1676) h0bf(1676:1678)
    cbf = np.zeros((H, 1678), bf)
    cbf[:, 0:768] = np.concatenate(
        [np.asarray(inputs["w_hh_f"]).T, np.asarray(inputs["w_hh_b"]).T], axis=1
    ).astype(bf)
    cbf[0:E, 768:1536] = np.concatenate(
        [np.asarray(inputs["w_ih_f"]).T, np.asarray(inputs["w_ih_b"]).T], axis=1
    ).astype(bf)
    w_out = np.asarray(inputs["w_out"], dtype=np.float32)  # [T, 2H]
    cbf[:, 1536:1548] = np.concatenate(
        [w_out[:, 0:H].T, w_out[:, H:2 * H].T], axis=1).astype(bf)
    cbf[:, 1548:1676] = np.eye(128, dtype=np.float32).astype(bf)
    cbf[:, 1676:1678] = np.stack([h0[0, 0], h0[1, 0]], axis=1).astype(bf)

    return {
        "sentence": sent,
        "emb": emb,
        "constf32": cf,
        "constbf": np.ascontiguousarray(cbf),
    }


def kernel(**inputs):
    global _last_results
    nc = bacc.Bacc("TRN2", target_bir_lowering=False, debug=False,
                   enable_asserts=True, num_devices=N_CORES)
    _build(nc)
    nc.compile()
    in_map = _prep_inputs(inputs)
    res = bass_utils.run_bass_kernel_spmd(
        nc, [in_map] * N_CORES, core_ids=list(range(N_CORES)),
        trace=bool(int(os.environ.get("KERNEL_TRACE", "0"))),
    )
    _last_results = res
    out = res.results[0]
    path = np.asarray(out["path"]).reshape(-1).astype(np.int32)
    score = np.asarray(out["score"]).reshape(()).astype(np.float32)
    return path, score


# revision 27
# speedup vs baseline: 1.1180x; 1.1180x over previous
"""Trainium2 Bass kernel for nn_BiLSTM_CRF (BiGRU + degenerate-CRF viterbi).

Key insight: the reference viterbi broadcasts prev over axis 0 and reduces over
axis 0, so new[j] = logit[j] + prev[j] + max_i trans[i,j] -- the recurrence
decouples across tags. Backpointers are time-constant A[j] = argmax_i
trans[i,j], path[t] = A^(L-t)(start), path[0] = 0, start = argmax_j of
(sum_t probs[t,j] + (L-1) * max_i trans[i,j]).

So the device pipeline is: embedding gather -> BiGRU (sequential, bf16
matmuls) -> logits softmax -> per-tag prob sums -> tiny map-iteration tail.
"""

import os

import numpy as np
import ml_dtypes

import concourse.bass as bass
import concourse.bacc as bacc
import concourse.tile as tile
from concourse import mybir
from concourse import bass_utils

V, E, H, T = 200000, 100, 128, 6
L = int(os.environ.get("KERNEL_L", "4096"))
UNROLL = int(os.environ.get("KERNEL_UNROLL", "0"))  # 0 = full static unroll
N_CORES = int(os.environ.get("KERNEL_CORES", "8"))

f32 = mybir.dt.float32
bf16 = mybir.dt.bfloat16
i32 = mybir.dt.int32
AF = mybir.ActivationFunctionType
OP = mybir.AluOpType
AX = mybir.AxisListType

_last_results = None


def _rev_free(ap):
    """Reverse the innermost free dim of a 2-D [P, W] AP slice."""
    w = ap.ap[-1][1]
    assert ap.ap[-1][0] == 1
    return bass.AP(tensor=ap.tensor, offset=ap.offset + (w - 1),
                   ap=[ap.ap[0], [-1, w]])


def _build(nc):
    # ---------------- DRAM I/O ----------------
    sent = nc.dram_tensor("sentence", [L], i32, kind="ExternalInput")
    emb = nc.dram_tensor("emb", [V, E], f32, kind="ExternalInput")
    # all small constants consolidated into two tensors -> two DMAs -> two
    # DMA-lane semaphores that each engine absorbs once (1-wait limit).
    cf_d = nc.dram_tensor("constf32", [H, 33], f32, kind="ExternalInput")
    cb_d = nc.dram_tensor("constbf", [H, 1935], bf16, kind="ExternalInput")

    path_out = nc.dram_tensor("path", [1, L + 1], i32, kind="ExternalOutput")
    score_out = nc.dram_tensor("score", [1, 1], f32, kind="ExternalOutput")

    NCH = L // 128          # gather / logits chunks
    NGI = L // 512          # gi matmul chunks

    with tile.TileContext(nc) as tc:
        with (
            tc.tile_pool(name="const", bufs=1) as cpool,
            tc.tile_pool(name="big", bufs=1) as big,
        ):
            # ------------- constants into SBUF -------------
            cf = cpool.tile([H, 33], f32)
            nc.sync.dma_start(out=cf[:], in_=cf_d[:, :])
            cb = cpool.tile([H, 1935], bf16)
            nc.sync.dma_start(out=cb[:], in_=cb_d[:, :])
            whhT = cb[:, 0:768]
            wihT = cb[:, 768:1536]
            woutT = cb[:, 1536:1548]
            id128 = cb[:, 1548:1676]
            h0bf = cb[:, 1676:1678]
            diagn = cb[:, 1678:1934]      # diag(b_hh_n) fwd then bwd
            onecol = cb[:, 1934:1935]
            gibias = cf[:, 0:6]
            bhhn = cf[:, 6:8]
            transT = cf[0:T, 14:20]
            iota6 = cf[0:1, 20:26]
            iota6r = cf[0:T, 20:26]
            id6 = cf[0:T, 26:32]
            ones128 = cf[:, 32:33]
            # DVE-local copies: TT-class DVE ops can encode only ONE sem wait,
            # so any tile feeding a tensor_tensor op must be DVE-produced.
            iota6v = cpool.tile([1, T], f32)
            nc.vector.tensor_copy(iota6v[:], iota6)
            iota6rv = cpool.tile([T, T], f32)
            nc.vector.tensor_copy(iota6rv[:], iota6r)
            boutv = cpool.tile([H, T], f32)
            nc.vector.tensor_copy(boutv[:], cf[:, 8:14])


            # ------------- big persistent buffers -------------
            xT = big.tile([128, L], bf16)           # x^T (E on partitions)
            gi_f = big.tile([H, 3 * L], bf16)       # input projections fwd
            gi_b = big.tile([H, 3 * L], bf16)       # input proj bwd, TIME-REVERSED
            hf = big.tile([H, L + 1], bf16)         # fwd hidden states
            hbrev = big.tile([H, L + 1], bf16)      # bwd hidden, reversed order
            hblin = big.tile([H, L], bf16)          # bwd hidden, original order

            nc.vector.tensor_copy(hf[:, 0:1], h0bf[:, 0:1])
            nc.vector.tensor_copy(hbrev[:, 0:1], h0bf[:, 1:2])

            # ------------- phase A: gather + transpose -------------
            with (
                tc.tile_pool(name="gather", bufs=NCH) as gpool,
                tc.tile_pool(name="trps", bufs=2, space="PSUM") as trps,
            ):
                for c in range(NCH):
                    idx = gpool.tile([128, 1], i32)
                    nc.sync.dma_start(out=idx, in_=sent[c * 128:(c + 1) * 128, None])
                    xg = gpool.tile([128, E], f32)
                    nc.gpsimd.indirect_dma_start(
                        out=xg[:],
                        out_offset=None,
                        in_=emb[:, :],
                        in_offset=bass.IndirectOffsetOnAxis(ap=idx[:, :1], axis=0),
                    )
                    xgb = gpool.tile([128, E], bf16)
                    nc.vector.tensor_copy(xgb[:], xg[:])
                    pt = trps.tile([E, 128], bf16)
                    nc.tensor.transpose(out=pt[:], in_=xgb[:], identity=id128)
                    nc.vector.tensor_copy(xT[:E, c * 128:(c + 1) * 128], pt[:, :])

            # ------------- phase B: gi = x @ WihT (+bias) -------------
            with tc.tile_pool(name="gips", bufs=2, space="PSUM") as gips:
                for dg in range(6):          # (dir, gate)
                    d, g = divmod(dg, 3)
                    for n in range(NGI):
                        pg = gips.tile([H, 512], f32)
                        nc.tensor.matmul(
                            out=pg[:],
                            lhsT=wihT[:E, dg * H:(dg + 1) * H],
                            rhs=xT[:E, n * 512:(n + 1) * 512],
                            start=True, stop=True,
                        )
                        if d == 0:
                            dst = gi_f[:, g * L + n * 512: g * L + (n + 1) * 512]
                        else:
                            # bwd step s handles original time L-1-s: write
                            # time chunk [512n, 512n+512) to reversed position
                            dst = _rev_free(
                                gi_b[:, g * L + L - 512 * (n + 1):
                                     g * L + L - 512 * n])
                        if g == 2:
                            # n-gate input projection is stored NEGATED so the
                            # recurrence can compute -n on ACT (see gru_step)
                            nc.vector.tensor_scalar(
                                dst, pg[:], gibias[:, dg:dg + 1], -1.0,
                                OP.add, OP.mult,
                            )
                        else:
                            nc.vector.tensor_scalar(
                                dst, pg[:], gibias[:, dg:dg + 1], None, OP.add,
                            )

            # ------------- phase C: the BiGRU recurrence -------------
            with (
                tc.tile_pool(name="psF", bufs=2, space="PSUM") as psF,
                tc.tile_pool(name="psB", bufs=2, space="PSUM") as psB,
                tc.tile_pool(name="step", bufs=4) as sp,
            ):
                def gru_step(t_f, whh_off, gi, hbuf, ps_pool, bhh_col, rd, wr):
                    """One GRU step; rd/wr are h-buffer column indices.

                    Engine layout tuned so every instruction carries at most
                    one new cross-engine dependency: PE does 4 matmuls (a
                    dependency-free diag(b_hh_n) seed + 3 gate matvecs), ACT
                    chains sigmoid-r, sigmoid-z, tmp=r*ps_n, -n=tanh(...) in
                    FIFO order, DVE finishes with dd=h-n and the h' update.
                    """
                    ps = ps_pool.tile([H, 3], f32, tag="ps")
                    # seed ps[:,2] with b_hh_n: constant inputs, issues early
                    nc.tensor.matmul(
                        out=ps[:, 2:3],
                        lhsT=diagn[:, bhh_col * H:(bhh_col + 1) * H],
                        rhs=onecol,
                        start=True, stop=False,
                    )
                    for g in range(3):
                        nc.tensor.matmul(
                            out=ps[:, g:g + 1],
                            lhsT=whhT[:, whh_off + g * H: whh_off + (g + 1) * H],
                            rhs=hbuf[:, rd],
                            start=(g != 2), stop=True,
                        )
                    rz = sp.tile([H, 2], f32, tag="rz")
                    nc.scalar.activation(rz[:, 0:1], ps[:, 0:1], AF.Sigmoid,
                                         bias=gi[:, bass.ds(t_f, 1)])
                    nc.scalar.activation(rz[:, 1:2], ps[:, 1:2], AF.Sigmoid,
                                         bias=gi[:, bass.ds(L + t_f, 1)])
                    # tmp = r * (h_n + b_hh_n)  (ACT, FIFO after sigmoids)
                    tmp = sp.tile([H, 1], f32, tag="tmp")
                    nc.scalar.activation(tmp[:], ps[:, 2:3], AF.Identity,
                                         scale=rz[:, 0:1])
                    # gi_n is stored negated, so this yields -n = tanh(-tmp-i_n)
                    nneg = sp.tile([H, 1], f32, tag="nneg")
                    nc.scalar.activation(nneg[:], tmp[:], AF.Tanh,
                                         bias=gi[:, bass.ds(2 * L + t_f, 1)],
                                         scale=-1.0)
                    # dd = h - n  (DVE; single ACT dep)
                    dd = sp.tile([H, 1], f32, tag="dd")
                    nc.vector.tensor_tensor(dd[:], hbuf[:, rd], nneg[:, 0:1],
                                            OP.add)
                    # h' = z*(h-n) + n = z*dd - nneg  (DVE FIFO)
                    nc.vector.tensor_scalar(hbuf[:, wr], dd[:],
                                            rz[:, 1:2], nneg[:, 0:1],
                                            OP.mult, OP.subtract)

                def both_steps(t):
                    # forward step t: h[t] -> h[t+1]
                    gru_step(t, 0, gi_f, hf, psF, 0,
                             bass.ds(t, 1), bass.ds(t + 1, 1))
                    # backward step t processes original time L-1-t, but
                    # all its buffers are stored time-reversed, so it
                    # indexes exactly like the forward direction.
                    gru_step(t, 3 * H, gi_b, hbrev, psB, 1,
                             bass.ds(t, 1), bass.ds(t + 1, 1))

                if UNROLL == 0:
                    for t in range(L):      # full static unroll
                        both_steps(t)
                else:
                    with tc.For_i(0, L, UNROLL,
                                  hint_engines=tuple(mybir.ALL_ENGINES)) as iv:
                        for k in range(UNROLL):
                            both_steps(iv + k)
            # restore original time order: h_b[t] = hbrev[:, L - t]
            nc.vector.tensor_copy(hblin[:], _rev_free(hbrev[:, 1:L + 1]))

            # ------------- phase D: logits, softmax, tag sums -------------
            Pacc = big.tile([H, T], f32)
            nc.vector.memset(Pacc[:], 0.0)
            with (
                tc.tile_pool(name="lps", bufs=2, space="PSUM") as lps,
                tc.tile_pool(name="lsb", bufs=3) as lsb,
            ):
                for c in range(NCH):
                    pl = lps.tile([H, T], f32)
                    nc.tensor.matmul(out=pl[:], lhsT=hf[:, 1 + c * 128: 1 + (c + 1) * 128],
                                     rhs=woutT[:, 0:T], start=True, stop=False)
                    nc.tensor.matmul(out=pl[:], lhsT=hblin[:, c * 128:(c + 1) * 128],
                                     rhs=woutT[:, T:2 * T], start=False, stop=True)
                    lg = lsb.tile([H, T], f32, tag="lg")
                    nc.vector.tensor_add(lg[:], pl[:], boutv[:])
                    ex = lsb.tile([H, T], f32, tag="ex")
                    se = lsb.tile([H, 1], f32, tag="se")
                    nc.scalar.activation(ex[:], lg[:], AF.Exp, accum_out=se[:])
                    rec = lsb.tile([H, 1], f32, tag="rec")
                    nc.vector.reciprocal(rec[:], se[:])
                    pb = lsb.tile([H, T], f32, tag="pb")
                    nc.vector.tensor_scalar(pb[:], ex[:], rec[:], None, OP.mult)
                    nc.vector.tensor_add(Pacc[:], Pacc[:], pb[:])

            # partition reduce S[j] = sum_t probs[t, j] via ones-matmul
            with tc.tile_pool(name="sps", bufs=1, space="PSUM") as sps:
                pS = sps.tile([1, T], f32)
                nc.tensor.matmul(out=pS[:], lhsT=ones128, rhs=Pacc[:],
                                 start=True, stop=True)
                S6 = big.tile([1, T], f32)
                nc.vector.tensor_copy(S6[:], pS[:])

            # ------------- phase E: M, A from transitions -------------
            Mcol = big.tile([T, 1], f32)
            nc.vector.tensor_reduce(Mcol[:], transT, AX.X, OP.max)
            eqT = big.tile([T, T], f32)
            nc.vector.tensor_scalar(eqT[:], transT, Mcol[:, 0:1], None, OP.is_equal)
            junk = big.tile([T, T], f32)
            nc.vector.tensor_mul(junk[:], eqT[:], iota6rv[:])
            Acol = big.tile([T, 1], f32)
            nc.vector.tensor_reduce(Acol[:], junk[:], AX.X, OP.max)
            with tc.tile_pool(name="maps", bufs=2, space="PSUM") as maps:
                pm = maps.tile([1, T], f32, tag="pm")
                nc.tensor.transpose(out=pm[:], in_=Mcol[:], identity=id6)
                Mf = big.tile([1, T], f32)
                nc.vector.tensor_copy(Mf[:], pm[:])
                pa = maps.tile([1, T], f32, tag="pa")
                nc.tensor.transpose(out=pa[:], in_=Acol[:], identity=id6)
                Af = big.tile([1, T], f32)
                nc.vector.tensor_copy(Af[:], pa[:])

            # ------------- phase F: score and start -------------
            fin = big.tile([1, T], f32)
            nc.vector.tensor_scalar(fin[:], Mf[:], float(L - 1), None, OP.mult)
            nc.vector.tensor_add(fin[:], fin[:], S6[:])
            sc = big.tile([1, 1], f32)
            nc.vector.tensor_reduce(sc[:], fin[:], AX.X, OP.max)
            eqf = big.tile([1, T], f32)
            nc.vector.tensor_scalar(eqf[:], fin[:], sc[:, 0:1], None, OP.is_equal)
            junk1 = big.tile([1, T], f32)
            nc.vector.tensor_mul(junk1[:], eqf[:], iota6v[:])
            st = big.tile([1, 1], f32)
            nc.vector.tensor_reduce(st[:], junk1[:], AX.X, OP.max)
            nc.sync.dma_start(out=score_out[:, :], in_=sc[:])

            # ------------- phase G: path via leftward doubling -------------
            # B[L-k] = A^k(start) for k=1..L-1; path[t] = B[t] for t=1..L-1.
            B = big.tile([1, L + 1], f32)
            C = big.tile([1, T], f32)       # C = A^m as a table
            nc.vector.tensor_copy(C[:], Af[:])
            # B[L-1] = A[start]
            selp = big.tile([1, T], f32)
            nc.vector.tensor_scalar(selp[:], iota6v[:], st[:, 0:1], None, OP.is_equal)
            junk2 = big.tile([1, T], f32)
            nc.vector.tensor_mul(junk2[:], selp[:], C[:])
            nc.vector.tensor_reduce(B[:, L - 1:L], junk2[:], AX.X, OP.max)
            with tc.tile_pool(name="dbl", bufs=2) as dbl:
                m = 1
                while m < L - 1:
                    ext = min(m, L - 1 - m)
                    # B[L-m-i] = C[B[L-i]] for i=1..ext
                    src = B[:, L - ext:L]
                    acc = dbl.tile([1, ext], f32, tag="acc")
                    nc.vector.memset(acc[:], 0.0)
                    for j in range(T):
                        mj = dbl.tile([1, ext], f32, tag="mj")
                        nc.vector.tensor_scalar(mj[:], src, float(j), None,
                                                OP.is_equal)
                        nc.vector.tensor_scalar(mj[:], mj[:], C[:, j:j + 1], None,
                                                OP.mult)
                        nc.vector.tensor_add(acc[:], acc[:], mj[:])
                    nc.vector.tensor_copy(B[:, L - m - ext:L - m], acc[:])
                    if 2 * m < L - 1:
                        # C = C o C
                        c2 = dbl.tile([1, T], f32, tag="c2")
                        nc.vector.memset(c2[:], 0.0)
                        for j in range(T):
                            cj = dbl.tile([1, T], f32, tag="cj")
                            nc.vector.tensor_scalar(cj[:], C[:], float(j), None,
                                                    OP.is_equal)
                            nc.vector.tensor_scalar(cj[:], cj[:], C[:, j:j + 1],
                                                    None, OP.mult)
                            nc.vector.tensor_add(c2[:], c2[:], cj[:])
                        nc.vector.tensor_copy(C[:], c2[:])
                    m += ext

            # ------------- phase H: assemble path -------------
            pf32 = big.tile([1, L + 1], f32)
            nc.vector.memset(pf32[:, 0:1], 0.0)
            nc.vector.tensor_copy(pf32[:, 1:L], B[:, 1:L])
            nc.vector.tensor_copy(pf32[:, L:L + 1], st[:])
            pi32 = big.tile([1, L + 1], i32)
            nc.vector.tensor_copy(pi32[:], pf32[:])
            nc.sync.dma_start(out=path_out[:, :], in_=pi32[:])

    return nc


def _prep_inputs(inputs):
    bf = ml_dtypes.bfloat16
    sent = np.ascontiguousarray(np.asarray(inputs["sentence"]).astype(np.int32))
    emb = np.ascontiguousarray(np.asarray(inputs["emb"], dtype=np.float32))
    h0 = np.asarray(inputs["h0"], dtype=np.float32)

    # constf32 [128, 33]: gibias(0:6) bhhn(6:8) boutrep(8:14) transT(14:20)
    # iota6rep(20:26) ident6(26:32) ones(32:33)
    cf = np.zeros((H, 33), np.float32)
    for d, (bih, bhh) in enumerate(
        [(inputs["b_ih_f"], inputs["b_hh_f"]), (inputs["b_ih_b"], inputs["b_hh_b"])]
    ):
        bih = np.asarray(bih, dtype=np.float32)
        bhh = np.asarray(bhh, dtype=np.float32)
        cf[:, d * 3 + 0] = bih[0:H] + bhh[0:H]
        cf[:, d * 3 + 1] = bih[H:2 * H] + bhh[H:2 * H]
        cf[:, d * 3 + 2] = bih[2 * H:3 * H]
        cf[:, 6 + d] = bhh[2 * H:3 * H]
    cf[:, 8:14] = np.asarray(inputs["b_out"], dtype=np.float32)[None, :]
    cf[0:T, 14:20] = np.asarray(inputs["transitions"], dtype=np.float32).T
    cf[0:T, 20:26] = np.arange(T, dtype=np.float32)[None, :]
    cf[0:T, 26:32] = np.eye(T, dtype=np.float32)
    cf[:, 32] = 1.0

    # constbf [128, 1678]: whhT(0:768) wihT(768:1536) woutT(1536:1548)
    # ident128(1548:1676) h0bf(1676:1678)
    cbf = np.zeros((H, 1935), bf)
    cbf[:, 0:768] = np.concatenate(
        [np.asarray(inputs["w_hh_f"]).T, np.asarray(inputs["w_hh_b"]).T], axis=1
    ).astype(bf)
    cbf[0:E, 768:1536] = np.concatenate(
        [np.asarray(inputs["w_ih_f"]).T, np.asarray(inputs["w_ih_b"]).T], axis=1
    ).astype(bf)
    w_out = np.asarray(inputs["w_out"], dtype=np.float32)  # [T, 2H]
    cbf[:, 1536:1548] = np.concatenate(
        [w_out[:, 0:H].T, w_out[:, H:2 * H].T], axis=1).astype(bf)
    cbf[:, 1548:1676] = np.eye(128, dtype=np.float32).astype(bf)
    cbf[:, 1676:1678] = np.stack([h0[0, 0], h0[1, 0]], axis=1).astype(bf)
    cbf[:, 1678:1806] = np.diag(
        np.asarray(inputs["b_hh_f"], dtype=np.float32)[2 * H:3 * H]).astype(bf)
    cbf[:, 1806:1934] = np.diag(
        np.asarray(inputs["b_hh_b"], dtype=np.float32)[2 * H:3 * H]).astype(bf)
    cbf[:, 1934] = 1.0

    return {
        "sentence": sent,
        "emb": emb,
        "constf32": cf,
        "constbf": np.ascontiguousarray(cbf),
    }


def kernel(**inputs):
    global _last_results
    nc = bacc.Bacc("TRN2", target_bir_lowering=False, debug=False,
                   enable_asserts=True, num_devices=N_CORES)
    _build(nc)
    nc.compile()
    in_map = _prep_inputs(inputs)
    res = bass_utils.run_bass_kernel_spmd(
        nc, [in_map] * N_CORES, core_ids=list(range(N_CORES)),
        trace=bool(int(os.environ.get("KERNEL_TRACE", "0"))),
    )
    _last_results = res
    out = res.results[0]
    path = np.asarray(out["path"]).reshape(-1).astype(np.int32)
    score = np.asarray(out["score"]).reshape(()).astype(np.float32)
    return path, score
